# revision 8
# baseline (speedup 1.0000x reference)
"""HardAttention kernel for Trainium2 (8 NeuronCores, Bass/Tile).

reference:
    scores = einsum("btd,bcsd->btcs", xs, ys)   # (B,Tx,C,Ty)
    out    = scores.max(-1).sum(1)              # (B,C)

Shapes: B=16, Tx=128, C=64, Ty=128, d=768.

Strategy:
  - Data-parallel over B: core i handles batches [2i, 2i+2).
  - Host pre-arranges both operands d-major (PE contracts over the
    partition axis) and pre-casts to fp8e4 (e4m3), so every device DMA is
    a same-dtype HWDGE transfer with fully contiguous partition lines:
        xsP[p, b, k, t]       = xs[b, t, 128k+p]        (128, B, KC, TX)
        ysP[b, q, p, k, c, s] = ys[b, 16q+c, s, 128k+p] (B, NQ, 128, KC, QC, TY)
  - Per (b, quarter-of-16-candidates): one contiguous slab DMA
    (128 partitions x KC*QC*TY elements), then fp8 DoubleRow matmuls
    (K_eff=256 per pass, 3 passes over d) into 4 PSUM banks; DVE
    reduce_max over Ty per candidate into an SBUF tile m[t, b, c];
    finally a single ones-vector matmul contracts the partition axis
    (sum over t) -> out[b, c].
"""

import os
import numpy as np
import ml_dtypes

B, TX, C, TY, D = 16, 128, 64, 128, 768
N_CORES = 8
BPC = B // N_CORES          # batches per core = 2
KC = D // 128               # contraction chunks = 6
QC = 16                     # candidates per slab
NQ = C // QC                # slabs per batch = 4
G = 4                       # candidates per matmul (N = G*TY = 512)

# "fp8": e4m3 operands, DoubleRow matmuls (2 d-chunks per pass).
# "bf16": bfloat16 operands, plain matmuls. Fallback if fp8 misbehaves.
MODE = os.environ.get("HA_MODE", "fp8")

_CACHE = {}
LAST_RESULT = None  # BassKernelResults of the most recent device run


def _np_dt():
    return ml_dtypes.float8_e4m3 if MODE == "fp8" else ml_dtypes.bfloat16


def _build():
    import concourse.bass as bass
    import concourse.mybir as mybir
    import concourse.tile as tile
    from concourse import bacc

    mm_dt = mybir.dt.float8e4 if MODE == "fp8" else mybir.dt.bfloat16
    f32 = mybir.dt.float32

    nc = bacc.Bacc(
        "TRN2",
        target_bir_lowering=False,
        debug=False,
        num_devices=N_CORES,
    )

    xs_ap = nc.dram_tensor("xsP", (128, BPC, KC, TX), mm_dt, kind="ExternalInput").ap()
    ys_ap = nc.dram_tensor(
        "ysP", (BPC, NQ, 128, KC, QC, TY), mm_dt, kind="ExternalInput"
    ).ap()
    out_ap = nc.dram_tensor("out", (1, BPC * C), f32, kind="ExternalOutput").ap()

    with tile.TileContext(nc) as tc:
        with (
            tc.tile_pool(name="xt", bufs=1) as xpool,
            tc.tile_pool(name="yt", bufs=16) as ypool,
            tc.tile_pool(name="mt", bufs=1) as mpool,
            tc.tile_pool(name="ones", bufs=1) as opool,
            tc.tile_pool(name="osb", bufs=1) as obpool,
            tc.tile_pool(name="ps", bufs=7, space="PSUM") as pspool,
            tc.tile_pool(name="pso", bufs=1, space="PSUM") as psopool,
        ):
            # All of xsP for this core: [p, b, k, t], fully contiguous.
            xt = xpool.tile([128, BPC, KC, TX], mm_dt)
            nc.scalar.dma_start(xt[:], xs_ap[:])

            ones = opool.tile([128, 1], mybir.dt.bfloat16)
            nc.any.memset(ones[:], 1.0)

            # max_s scores, [t, b, c]; bf16 so the final t-sum matmul is a
            # single-instruction bf16 op (max |err| ~0.4%, within tolerance)
            m = mpool.tile([128, BPC, C], mybir.dt.bfloat16)

            qeng = [nc.sync, nc.scalar]
            nq_dma = 0
            KP = KC // 2  # k-pair chunks per slab = 3
            for b in range(BPC):
                for q in range(NQ):
                    # k-pair chunk DMAs: [p, 2, c, s] — contiguous partition
                    # lines, fine-grained so matmuls start after the first
                    # 0.5MB instead of the full 1.6MB slab. The very last
                    # slab is additionally split along candidates so its
                    # reduce chain overlaps its own DMA tail.
                    last = b == BPC - 1 and q == NQ - 1
                    csplits = (
                        [(0, 8), (8, 4), (12, 4)]
                        if (last and MODE == "fp8")
                        else [(0, QC)]
                    )
                    if MODE == "fp8":
                        for c0, nch in csplits:
                            ngr = nch // G
                            yts = []
                            for kk in range(KP):
                                yt = ypool.tile([128, 2, nch, TY], mm_dt)
                                qeng[nq_dma % 2].dma_start(
                                    yt[:],
                                    ys_ap[
                                        b, q, :, 2 * kk : 2 * kk + 2, c0 : c0 + nch
                                    ],
                                )
                                nq_dma += 1
                                yts.append(yt)
                            psums = [
                                pspool.tile(
                                    [128, G, TY],
                                    f32,
                                    name=f"ps_{b}_{q}_{c0}_{gi}",
                                    tag="ps",
                                )
                                for gi in range(ngr)
                            ]
                            # kk-outer: after the last k-pair tile lands, only
                            # the stop-matmuls remain on the in-order PE queue.
                            for kk in range(KP):
                                for gi in range(ngr):
                                    nc.tensor.matmul(
                                        psums[gi][:],
                                        lhsT=xt[:, b, 2 * kk : 2 * kk + 2, :],
                                        rhs=yts[kk][:, :, gi * G : (gi + 1) * G, :],
                                        start=(kk == 0),
                                        stop=(kk == KP - 1),
                                        perf_mode=mybir.MatmulPerfMode.DoubleRow,
                                    )
                            for gi in range(ngr):
                                cc = q * QC + c0 + gi * G
                                nc.vector.reduce_max(
                                    m[:, b, cc : cc + G],
                                    psums[gi][:],
                                    axis=mybir.AxisListType.X,
                                )
                    else:
                        yt = ypool.tile([128, KC, QC, TY], mm_dt)
                        qeng[nq_dma % 2].dma_start(yt[:], ys_ap[b, q])
                        nq_dma += 1
                        psums = [
                            pspool.tile(
                                [128, G, TY], f32, name=f"ps_{b}_{q}_{g}", tag="ps"
                            )
                            for g in range(G)
                        ]
                        for k in range(KC):
                            for g in range(G):
                                nc.tensor.matmul(
                                    psums[g][:],
                                    lhsT=xt[:, b, k, :],
                                    rhs=yt[:, k, g * G : (g + 1) * G, :],
                                    start=(k == 0),
                                    stop=(k == KC - 1),
                                )
                        for g in range(G):
                            cc = q * QC + g * G
                            nc.vector.reduce_max(
                                m[:, b, cc : cc + G],
                                psums[g][:],
                                axis=mybir.AxisListType.X,
                            )
                # sum over t (partition axis) via ones-vector matmul; per-b so
                # only the last batch's (tiny) chain is on the critical tail
                out_ps = psopool.tile([1, C], f32, tag="out_ps")
                nc.tensor.matmul(
                    out_ps[:], lhsT=ones[:], rhs=m[:, b, :], start=True, stop=True
                )
                osb = obpool.tile([1, C], f32, tag="osb")
                nc.vector.tensor_copy(osb[:], out_ps[:])
                nc.sync.dma_start(out_ap[0, b * C : (b + 1) * C], osb[:])

    nc.compile()
    return nc


def _get_nc():
    key = ("nc", MODE)
    if key not in _CACHE:
        _CACHE[key] = _build()
    return _CACHE[key]


def _prep(xs: np.ndarray, ys: np.ndarray):
    """Host-side layout: d-major, blocked by 128-chunks of d, cast to mm dtype."""
    np_dt = _np_dt()
    xs = np.asarray(xs, dtype=np.float32)
    ys = np.asarray(ys, dtype=np.float32)
    # xsP[p, b, k, t] = xs[b, t, 128k+p]
    xsP = np.ascontiguousarray(
        xs.astype(np_dt).reshape(B, TX, KC, 128).transpose(3, 0, 2, 1)
    )
    # ysP[b, q, p, k, c, s] = ys[b, 16q+c, s, 128k+p]
    ysP = np.ascontiguousarray(
        ys.astype(np_dt).reshape(B, NQ, QC, TY, KC, 128).transpose(0, 1, 5, 4, 2, 3)
    )
    return xsP, ysP


def kernel(xs: np.ndarray, ys: np.ndarray) -> np.ndarray:
    global LAST_RESULT
    from concourse.bass_utils import run_bass_kernel_spmd

    nc = _get_nc()
    xsP, ysP = _prep(xs, ys)
    in_maps = [
        {
            "xsP": np.ascontiguousarray(xsP[:, i * BPC : (i + 1) * BPC]),
            "ysP": ysP[i * BPC : (i + 1) * BPC],
        }
        for i in range(N_CORES)
    ]
    res = run_bass_kernel_spmd(nc, in_maps, core_ids=list(range(N_CORES)))
    LAST_RESULT = res
    out = np.concatenate(
        [res.results[i]["out"].reshape(BPC, C) for i in range(N_CORES)], axis=0
    )
    return out.astype(np.float32)
